# revision 22
# baseline (speedup 1.0000x reference)
"""Bass/Trainium2 kernel for nn_Parser (gnn_message_passing).

The only O(nmol*molsize^2) work in the reference is the dense pairwise
distance pass + cutoff mask; everything else is O(atoms + pairs) index
arithmetic done on host in numpy (bitwise-identical to the jax reference).

Device kernel (SPMD over 8 cores, 64 molecules each), built for an
environment where every engine instruction has a large fixed cost: the
whole kernel is ~10 instructions per core (1 input DMA, 5 vector-engine
ops, 1 output DMA, 3 waits).

  Layout: partition p = (molecule m = p//2, i-half h = p%2). Each
  partition holds its molecule's full coordinates (for j) plus its
  i-half slice, so the pairwise difference needs no cross-partition
  broadcast:
    sub    : D3[p, i', j, c] = X[m, j, c] - X[m, h*64+i', c]  (one op,
             0-stride free dims on both operands)
    mult   : D3 *= D3                                         (in place)
    reduce : D2[p, i', j] = sum_c D3                          (axis X)
    stt    : MW = (D2 < 145.0) * 2^(j%8)   (fused compare + bit weight,
             scalar_tensor_tensor with is_lt/mult)
    reduce : pk byte = sum over 8-groups of MW  (= little-endian
             packbits, 1 bit/pair -> [128, 1024] u8 per core)
  Blank atoms are pre-moved to distinct far-away coordinates so every
  pair involving one fails the cutoff. i>=j and the loose 145 cutoff are
  filtered on host, where candidates are re-checked with the exact f32
  d2 < 144 (reproduces the reference compaction bitwise).
"""

import numpy as np

NMOL, MOLSIZE = 512, 128
N_CORES = 8
MPC = NMOL // N_CORES                   # 64 molecules per core
NPAIR = MPC * MOLSIZE * MOLSIZE         # pair grid per core
OUTERCUTOFF2 = 144.0
LOOSE_CUT = 145.0
LCF = 1.8897259886

_CACHE = {}


def _build_program(repeat=1):
    import concourse.bass as bass
    import concourse.mybir as mybir

    f32 = mybir.dt.float32
    u8 = mybir.dt.uint8
    H = MOLSIZE // 2                    # 64 rows of i per partition
    NF3 = H * MOLSIZE * 3               # 24576 free elems of D3
    NP = H * MOLSIZE                    # 8192 pairs per partition

    nc = bass.Bass(detect_race_conditions=False)
    # x2 row p: [full X of mol p//2 (384) | its i-half (192) | bit weights (8)]
    x2_d = nc.declare_dram_parameter("x2", [128, 584], f32, isOutput=False)
    pk_d = nc.declare_dram_parameter("pk", [128, NP // 8], u8, isOutput=True)

    with (
        nc.sbuf_tensor([128, 584], f32) as X2,
        nc.sbuf_tensor([128, NF3], f32) as D3,
        nc.sbuf_tensor([128, NP], f32) as D2,
        nc.sbuf_tensor([128, NP], u8) as MW,
        nc.sbuf_tensor([128, NP // 8], u8) as P3,
        nc.semaphore("s_in") as s_in,
        nc.semaphore("s_v") as s_v,
        nc.semaphore("s_o") as s_o,
        nc.Block() as block,
    ):
        @block.sync
        def _(eng):
            for r in range(repeat):
                eng.dma_start(out=X2[:], in_=x2_d[:]).then_inc(s_in, 16)
                eng.wait_ge(s_v, r + 1)
                eng.dma_start(out=pk_d[:], in_=P3[:]).then_inc(s_o, 16)
            eng.wait_ge(s_o, 16 * repeat)

        @block.vector
        def _(eng):
            for r in range(repeat):
                eng.wait_ge(s_in, 16 * (r + 1))
                xj = X2[:, 0:384].rearrange("p (j c) -> p j c", c=3)
                xj = xj.unsqueeze(1).broadcast_to([128, H, MOLSIZE, 3])
                xi = X2[:, 384:576].rearrange("p (i c) -> p i c", c=3)
                xi = xi.unsqueeze(2).broadcast_to([128, H, MOLSIZE, 3])
                d3 = D3.rearrange("p (i j c) -> p i j c", i=H, j=MOLSIZE)
                nc.vector.tensor_tensor(d3, xj, xi,
                                        op=mybir.AluOpType.subtract)
                nc.vector.tensor_mul(D3[:], D3[:], D3[:])
                nc.vector.reduce_sum(D2[:],
                                     D3.rearrange("p (n c) -> p n c", c=3),
                                     axis=mybir.AxisListType.X)
                # MW = (d2 < 145) * 2^(j%8), one fused op
                wts = X2[:, 576:584].unsqueeze(1)
                wts = wts.broadcast_to([128, NP // 8, 8])
                nc.vector.scalar_tensor_tensor(
                    out=MW.rearrange("p (n b) -> p n b", b=8),
                    in0=D2.rearrange("p (n b) -> p n b", b=8),
                    scalar=LOOSE_CUT, in1=wts,
                    op0=mybir.AluOpType.is_lt,
                    op1=mybir.AluOpType.mult,
                )
                # byte n = sum_b bit_b * 2^b  (little-endian packbits;
                # sum <= 255 so u8 accumulation is exact)
                with nc.allow_low_precision(reason="u8 bitpack sum <= 255"):
                    pack = nc.vector.reduce_sum(
                        P3[:], MW.rearrange("p (n b) -> p n b", b=8),
                        axis=mybir.AxisListType.X)
                pack.then_inc(s_v, 1)
    return nc


def _get_program(repeat=1):
    key = ("nc", repeat)
    if key not in _CACHE:
        _CACHE[key] = _build_program(repeat)
    return _CACHE[key]


def _device_inputs(species, coordinates):
    """Per-core input maps: x2[p] = [full X of mol p//2 | its (p%2) i-half]."""
    nb = species > 0                                         # [512,128]
    Xm = np.ascontiguousarray(coordinates, dtype=np.float32).copy()
    aidx = np.arange(MOLSIZE, dtype=np.float32)
    blank = np.zeros((MOLSIZE, 3), np.float32)
    blank[:, 0] = 1.0e5 + 1.0e3 * aidx
    Xm[~nb] = np.broadcast_to(blank[None], Xm.shape)[~nb]

    H = MOLSIZE // 2
    wts = np.broadcast_to((2.0 ** np.arange(8, dtype=np.float32))[None],
                          (128, 8))
    in_maps = []
    for c in range(N_CORES):
        Xc = Xm[c * MPC:(c + 1) * MPC]                       # [64,128,3]
        full = np.repeat(Xc.reshape(MPC, MOLSIZE * 3), 2, axis=0)  # [128,384]
        halves = Xc.reshape(MPC, 2, H * 3).reshape(128, H * 3)     # [128,192]
        x2 = np.concatenate([full, halves, wts], axis=1)     # [128,584]
        in_maps.append({"x2": np.ascontiguousarray(x2, dtype=np.float32)})
    return in_maps


def _run_device(species, coordinates, trace=False, repeat=1):
    from concourse.bass_utils import run_bass_kernel_spmd

    nc = _get_program(repeat)
    in_maps = _device_inputs(species, coordinates)
    res = run_bass_kernel_spmd(nc, in_maps, list(range(N_CORES)), trace=trace)
    _CACHE["last_res"] = res
    H = MOLSIZE // 2
    parts = []
    for c in range(N_CORES):
        pk = res.results[c]["pk"]                            # [128, 1024]
        bits = np.unpackbits(pk.reshape(MPC, 2, H, MOLSIZE // 8),
                             axis=-1, bitorder="little")     # [64,2,64,128]
        parts.append(bits.reshape(MPC, MOLSIZE, MOLSIZE))    # [64,i,j]
    cand = np.concatenate(parts, axis=0)                     # [512,128,128]
    return cand, res


def kernel(species, coordinates, tot_charge, tore, _trace=False):
    species = np.asarray(species)
    coordinates = np.ascontiguousarray(np.asarray(coordinates,
                                                  dtype=np.float32))
    tot_charge = np.asarray(tot_charge)
    tore = np.asarray(tore, dtype=np.float32)

    nmol, molsize = species.shape
    sp32 = species.astype(np.int32)

    cand, _ = _run_device(sp32, coordinates, trace=_trace)

    # ---- host post-processing (identical to reference, validated bitwise) ----
    nonblank = sp32 > 0
    flat_nb = nonblank.ravel()
    real_atoms = np.nonzero(flat_nb)[0].astype(np.int32)
    n_real = len(real_atoms)
    inv_real_atoms = np.zeros(nmol * molsize, np.int32)
    inv_real_atoms[real_atoms] = np.arange(n_real, dtype=np.int32)
    Z = sp32.ravel()[real_atoms]

    nHeavy = (sp32 > 1).sum(1).astype(np.int32)
    sp = sp32
    nSuperHeavy = (((sp > 12) & (sp < 18)) | ((sp > 20) & (sp < 30))
                   | ((sp > 32) & (sp < 36)) | ((sp > 38) & (sp < 48))
                   | ((sp > 50) & (sp < 54)) | ((sp > 70) & (sp < 80))
                   | (sp == 57)).sum(1).astype(np.int32)
    nHydro = (sp == 1).sum(1).astype(np.int32)

    n_charge = tore[sp32].sum(axis=1, dtype=np.float32).astype(np.int32) \
        - tot_charge.astype(np.int32)
    nocc = n_charge // 2

    t1 = (np.arange(molsize) * (molsize + 1)).reshape(1, -1)
    t2 = (np.arange(nmol) * molsize ** 2).reshape(-1, 1)
    maskd = (t1 + t2).ravel()[real_atoms].astype(np.int32)
    atom_molid = np.repeat(np.arange(nmol, dtype=np.int32), molsize)[flat_nb]

    # exact refilter of device candidates (device grid includes i>=j)
    mi, ii, jj = np.nonzero(cand)
    tri = ii < jj
    mi = mi[tri]; ii = ii[tri]; jj = jj[tri]
    ci = coordinates[mi, ii]
    cj = coordinates[mi, jj]
    pc = cj - ci
    p = pc * pc
    d2 = (p[:, 0] + p[:, 1]) + p[:, 2]
    keep = d2 < np.float32(OUTERCUTOFF2)
    mi = mi[keep]; ii = ii[keep]; jj = jj[keep]; pc = pc[keep]; p = p[keep]

    pairdist = np.sqrt((p[:, 0] + p[:, 1]) + p[:, 2]).astype(np.float32)
    rij = pairdist * np.float32(LCF)
    xij = pc / pairdist[:, None]

    pair_first_val = (mi * molsize + ii).astype(np.int32)
    pair_second_val = (mi * molsize + jj).astype(np.int32)
    idxi = inv_real_atoms[pair_first_val]
    idxj = inv_real_atoms[pair_second_val]
    ni = Z[idxi]
    nj = Z[idxj]
    mask_o = (real_atoms[idxi] * molsize
              + real_atoms[idxj] % molsize).astype(np.int32)
    mask_l = (real_atoms[idxj] * molsize
              + real_atoms[idxi] % molsize).astype(np.int32)
    pair_molid = atom_molid[idxi]

    return (nmol, molsize, nSuperHeavy, nHeavy, nHydro, nocc, Z, maskd,
            atom_molid, mask_o, mask_l, pair_molid, ni, nj, idxi, idxj,
            xij, rij)


# revision 24
# speedup vs baseline: 1.1619x; 1.1619x over previous
"""Bass/Trainium2 kernel for nn_Parser (gnn_message_passing).

The only O(nmol*molsize^2) work in the reference is the dense pairwise
distance pass + cutoff mask; everything else is O(atoms + pairs) index
arithmetic done on host in numpy (bitwise-identical to the jax reference).

Device kernel (SPMD over 8 cores, 64 molecules each), built for an
environment where every engine instruction has a large fixed cost: the
whole kernel is ~8 instructions per core (1 input DMA, 5 vector-engine
ops, 1 output DMA, 1 standalone wait; the other two waits are attached
to the first vector op and the output DMA).

  Layout: partition p = (molecule m = p//2, i-half h = p%2). Each
  partition holds its molecule's full coordinates (for j) plus its
  i-half slice, so the pairwise difference needs no cross-partition
  broadcast:
    sub    : D3[p, i', j, c] = X[m, j, c] - X[m, h*64+i', c]  (one op,
             0-stride free dims on both operands)
    mult   : D3 *= D3                                         (in place)
    reduce : D2[p, i', j] = sum_c D3                          (axis X)
    stt    : MW = (D2 < 145.0) * 2^(j%8)   (fused compare + bit weight,
             scalar_tensor_tensor with is_lt/mult)
    reduce : pk byte = sum over 8-groups of MW  (= little-endian
             packbits, 1 bit/pair -> [128, 1024] u8 per core)
  Blank atoms are pre-moved to distinct far-away coordinates so every
  pair involving one fails the cutoff. i>=j and the loose 145 cutoff are
  filtered on host, where candidates are re-checked with the exact f32
  d2 < 144 (reproduces the reference compaction bitwise).
"""

import numpy as np

NMOL, MOLSIZE = 512, 128
N_CORES = 8
MPC = NMOL // N_CORES                   # 64 molecules per core
NPAIR = MPC * MOLSIZE * MOLSIZE         # pair grid per core
OUTERCUTOFF2 = 144.0
LOOSE_CUT = 145.0
LCF = 1.8897259886

_CACHE = {}


def _build_program(repeat=1):
    import concourse.bass as bass
    import concourse.mybir as mybir

    f32 = mybir.dt.float32
    u8 = mybir.dt.uint8
    H = MOLSIZE // 2                    # 64 rows of i per partition
    NF3 = H * MOLSIZE * 3               # 24576 free elems of D3
    NP = H * MOLSIZE                    # 8192 pairs per partition

    nc = bass.Bass(detect_race_conditions=False)
    # x2 row p: [full X of mol p//2 (384) | its i-half (192) | bit weights (8)]
    x2_d = nc.declare_dram_parameter("x2", [128, 584], f32, isOutput=False)
    pk_d = nc.declare_dram_parameter("pk", [128, NP // 8], u8, isOutput=True)

    with (
        nc.sbuf_tensor([128, 584], f32) as X2,
        nc.sbuf_tensor([128, NF3], f32) as D3,
        nc.sbuf_tensor([128, NP], f32) as D2,
        nc.sbuf_tensor([128, NP], u8) as MW,
        nc.sbuf_tensor([128, NP // 8], u8) as P3,
        nc.semaphore("s_in") as s_in,
        nc.semaphore("s_v") as s_v,
        nc.semaphore("s_o") as s_o,
        nc.Block() as block,
    ):
        @block.sync
        def _(eng):
            for r in range(repeat):
                eng.dma_start(out=X2[:], in_=x2_d[:]).then_inc(s_in, 16)
                # the pre-output wait rides on the DMA itself (this walrus
                # allows exactly one attached wait per instruction)
                eng.dma_start(out=pk_d[:], in_=P3[:])._wait_ge(
                    s_v, r + 1).then_inc(s_o, 16)
            eng.wait_ge(s_o, 16 * repeat)

        @block.vector
        def _(eng):
            for r in range(repeat):
                xj = X2[:, 0:384].rearrange("p (j c) -> p j c", c=3)
                xj = xj.unsqueeze(1).broadcast_to([128, H, MOLSIZE, 3])
                xi = X2[:, 384:576].rearrange("p (i c) -> p i c", c=3)
                xi = xi.unsqueeze(2).broadcast_to([128, H, MOLSIZE, 3])
                d3 = D3.rearrange("p (i j c) -> p i j c", i=H, j=MOLSIZE)
                nc.vector.tensor_tensor(d3, xj, xi,
                                        op=mybir.AluOpType.subtract)._wait_ge(
                                            s_in, 16 * (r + 1))
                nc.vector.tensor_mul(D3[:], D3[:], D3[:])
                nc.vector.reduce_sum(D2[:],
                                     D3.rearrange("p (n c) -> p n c", c=3),
                                     axis=mybir.AxisListType.X)
                # MW = (d2 < 145) * 2^(j%8), one fused op
                wts = X2[:, 576:584].unsqueeze(1)
                wts = wts.broadcast_to([128, NP // 8, 8])
                nc.vector.scalar_tensor_tensor(
                    out=MW.rearrange("p (n b) -> p n b", b=8),
                    in0=D2.rearrange("p (n b) -> p n b", b=8),
                    scalar=LOOSE_CUT, in1=wts,
                    op0=mybir.AluOpType.is_lt,
                    op1=mybir.AluOpType.mult,
                )
                # byte n = sum_b bit_b * 2^b  (little-endian packbits;
                # sum <= 255 so u8 accumulation is exact)
                with nc.allow_low_precision(reason="u8 bitpack sum <= 255"):
                    pack = nc.vector.reduce_sum(
                        P3[:], MW.rearrange("p (n b) -> p n b", b=8),
                        axis=mybir.AxisListType.X)
                pack.then_inc(s_v, 1)
    return nc


def _get_program(repeat=1):
    key = ("nc", repeat)
    if key not in _CACHE:
        _CACHE[key] = _build_program(repeat)
    return _CACHE[key]


def _device_inputs(species, coordinates):
    """Per-core input maps: x2[p] = [full X of mol p//2 | its (p%2) i-half]."""
    nb = species > 0                                         # [512,128]
    Xm = np.ascontiguousarray(coordinates, dtype=np.float32).copy()
    aidx = np.arange(MOLSIZE, dtype=np.float32)
    blank = np.zeros((MOLSIZE, 3), np.float32)
    blank[:, 0] = 1.0e5 + 1.0e3 * aidx
    Xm[~nb] = np.broadcast_to(blank[None], Xm.shape)[~nb]

    H = MOLSIZE // 2
    wts = np.broadcast_to((2.0 ** np.arange(8, dtype=np.float32))[None],
                          (128, 8))
    in_maps = []
    for c in range(N_CORES):
        Xc = Xm[c * MPC:(c + 1) * MPC]                       # [64,128,3]
        full = np.repeat(Xc.reshape(MPC, MOLSIZE * 3), 2, axis=0)  # [128,384]
        halves = Xc.reshape(MPC, 2, H * 3).reshape(128, H * 3)     # [128,192]
        x2 = np.concatenate([full, halves, wts], axis=1)     # [128,584]
        in_maps.append({"x2": np.ascontiguousarray(x2, dtype=np.float32)})
    return in_maps


def _run_device(species, coordinates, trace=False, repeat=1):
    from concourse.bass_utils import run_bass_kernel_spmd

    nc = _get_program(repeat)
    in_maps = _device_inputs(species, coordinates)
    res = run_bass_kernel_spmd(nc, in_maps, list(range(N_CORES)), trace=trace)
    _CACHE["last_res"] = res
    H = MOLSIZE // 2
    parts = []
    for c in range(N_CORES):
        pk = res.results[c]["pk"]                            # [128, 1024]
        bits = np.unpackbits(pk.reshape(MPC, 2, H, MOLSIZE // 8),
                             axis=-1, bitorder="little")     # [64,2,64,128]
        parts.append(bits.reshape(MPC, MOLSIZE, MOLSIZE))    # [64,i,j]
    cand = np.concatenate(parts, axis=0)                     # [512,128,128]
    return cand, res


def kernel(species, coordinates, tot_charge, tore, _trace=False):
    species = np.asarray(species)
    coordinates = np.ascontiguousarray(np.asarray(coordinates,
                                                  dtype=np.float32))
    tot_charge = np.asarray(tot_charge)
    tore = np.asarray(tore, dtype=np.float32)

    nmol, molsize = species.shape
    sp32 = species.astype(np.int32)

    cand, _ = _run_device(sp32, coordinates, trace=_trace)

    # ---- host post-processing (identical to reference, validated bitwise) ----
    nonblank = sp32 > 0
    flat_nb = nonblank.ravel()
    real_atoms = np.nonzero(flat_nb)[0].astype(np.int32)
    n_real = len(real_atoms)
    inv_real_atoms = np.zeros(nmol * molsize, np.int32)
    inv_real_atoms[real_atoms] = np.arange(n_real, dtype=np.int32)
    Z = sp32.ravel()[real_atoms]

    nHeavy = (sp32 > 1).sum(1).astype(np.int32)
    sp = sp32
    nSuperHeavy = (((sp > 12) & (sp < 18)) | ((sp > 20) & (sp < 30))
                   | ((sp > 32) & (sp < 36)) | ((sp > 38) & (sp < 48))
                   | ((sp > 50) & (sp < 54)) | ((sp > 70) & (sp < 80))
                   | (sp == 57)).sum(1).astype(np.int32)
    nHydro = (sp == 1).sum(1).astype(np.int32)

    n_charge = tore[sp32].sum(axis=1, dtype=np.float32).astype(np.int32) \
        - tot_charge.astype(np.int32)
    nocc = n_charge // 2

    t1 = (np.arange(molsize) * (molsize + 1)).reshape(1, -1)
    t2 = (np.arange(nmol) * molsize ** 2).reshape(-1, 1)
    maskd = (t1 + t2).ravel()[real_atoms].astype(np.int32)
    atom_molid = np.repeat(np.arange(nmol, dtype=np.int32), molsize)[flat_nb]

    # exact refilter of device candidates (device grid includes i>=j)
    mi, ii, jj = np.nonzero(cand)
    tri = ii < jj
    mi = mi[tri]; ii = ii[tri]; jj = jj[tri]
    ci = coordinates[mi, ii]
    cj = coordinates[mi, jj]
    pc = cj - ci
    p = pc * pc
    d2 = (p[:, 0] + p[:, 1]) + p[:, 2]
    keep = d2 < np.float32(OUTERCUTOFF2)
    mi = mi[keep]; ii = ii[keep]; jj = jj[keep]; pc = pc[keep]; p = p[keep]

    pairdist = np.sqrt((p[:, 0] + p[:, 1]) + p[:, 2]).astype(np.float32)
    rij = pairdist * np.float32(LCF)
    xij = pc / pairdist[:, None]

    pair_first_val = (mi * molsize + ii).astype(np.int32)
    pair_second_val = (mi * molsize + jj).astype(np.int32)
    idxi = inv_real_atoms[pair_first_val]
    idxj = inv_real_atoms[pair_second_val]
    ni = Z[idxi]
    nj = Z[idxj]
    mask_o = (real_atoms[idxi] * molsize
              + real_atoms[idxj] % molsize).astype(np.int32)
    mask_l = (real_atoms[idxj] * molsize
              + real_atoms[idxi] % molsize).astype(np.int32)
    pair_molid = atom_molid[idxi]

    return (nmol, molsize, nSuperHeavy, nHeavy, nHydro, nocc, Z, maskd,
            atom_molid, mask_o, mask_l, pair_molid, ni, nj, idxi, idxj,
            xij, rij)


# revision 25
# speedup vs baseline: 1.1816x; 1.0170x over previous
"""Bass/Trainium2 kernel for nn_Parser (gnn_message_passing).

The only O(nmol*molsize^2) work in the reference is the dense pairwise
distance pass + cutoff mask; everything else is O(atoms + pairs) index
arithmetic done on host in numpy (bitwise-identical to the jax reference).

Device kernel (SPMD over 8 cores, 64 molecules each), built for an
environment where every engine instruction has a large fixed cost: the
whole kernel is ~8 instructions per core (1 input DMA, 5 vector-engine
ops, 1 output DMA, 1 standalone wait; the other two waits are attached
to the first vector op and the output DMA).

  Layout: partition p = (molecule m = p//2, i-half h = p%2). Each
  partition holds its molecule's full coordinates (for j) plus its
  i-half slice, so the pairwise difference needs no cross-partition
  broadcast:
    sub    : D3[p, i', j, c] = X[m, j, c] - X[m, h*64+i', c]  (one op,
             0-stride free dims on both operands)
    mult   : D3 *= D3                                         (in place)
    reduce : D2[p, i', j] = sum_c D3                          (axis X)
    stt    : MW = (D2 < 145.0) * 2^(j%8)   (fused compare + bit weight,
             scalar_tensor_tensor with is_lt/mult)
    reduce : pk byte = sum over 8-groups of MW  (= little-endian
             packbits, 1 bit/pair -> [128, 1024] u8 per core)
  Blank atoms are pre-moved to distinct far-away coordinates so every
  pair involving one fails the cutoff. i>=j and the loose 145 cutoff are
  filtered on host, where candidates are re-checked with the exact f32
  d2 < 144 (reproduces the reference compaction bitwise).
"""

import numpy as np

NMOL, MOLSIZE = 512, 128
N_CORES = 8
MPC = NMOL // N_CORES                   # 64 molecules per core
NPAIR = MPC * MOLSIZE * MOLSIZE         # pair grid per core
OUTERCUTOFF2 = 144.0
LOOSE_CUT = 148.0   # covers the f16-input perturbation bound (~2.5)
LCF = 1.8897259886

_CACHE = {}


def _build_program(repeat=1):
    import concourse.bass as bass
    import concourse.mybir as mybir

    f32 = mybir.dt.float32
    f16 = mybir.dt.float16
    u8 = mybir.dt.uint8
    H = MOLSIZE // 2                    # 64 rows of i per partition
    NF3 = H * MOLSIZE * 3               # 24576 free elems of D3
    NP = H * MOLSIZE                    # 8192 pairs per partition

    nc = bass.Bass(detect_race_conditions=False)
    # x2 row p: [full X of mol p//2 (384) | its i-half (192) | bit weights (8)]
    x2_d = nc.declare_dram_parameter("x2", [128, 584], f16, isOutput=False)
    pk_d = nc.declare_dram_parameter("pk", [128, NP // 8], u8, isOutput=True)

    with (
        nc.sbuf_tensor([128, 584], f16) as X2,
        nc.sbuf_tensor([128, NF3], f32) as D3,
        nc.sbuf_tensor([128, NP], f32) as D2,
        nc.sbuf_tensor([128, NP], u8) as MW,
        nc.sbuf_tensor([128, NP // 8], u8) as P3,
        nc.semaphore("s_in") as s_in,
        nc.semaphore("s_v") as s_v,
        nc.semaphore("s_o") as s_o,
        nc.Block() as block,
    ):
        @block.sync
        def _(eng):
            for r in range(repeat):
                eng.dma_start(out=X2[:], in_=x2_d[:]).then_inc(s_in, 16)
                # the pre-output wait rides on the DMA itself (this walrus
                # allows exactly one attached wait per instruction)
                eng.dma_start(out=pk_d[:], in_=P3[:])._wait_ge(
                    s_v, r + 1).then_inc(s_o, 16)
            eng.wait_ge(s_o, 16 * repeat)

        @block.vector
        def _(eng):
            for r in range(repeat):
                xj = X2[:, 0:384].rearrange("p (j c) -> p j c", c=3)
                xj = xj.unsqueeze(1).broadcast_to([128, H, MOLSIZE, 3])
                xi = X2[:, 384:576].rearrange("p (i c) -> p i c", c=3)
                xi = xi.unsqueeze(2).broadcast_to([128, H, MOLSIZE, 3])
                d3 = D3.rearrange("p (i j c) -> p i j c", i=H, j=MOLSIZE)
                nc.vector.tensor_tensor(d3, xj, xi,
                                        op=mybir.AluOpType.subtract)._wait_ge(
                                            s_in, 16 * (r + 1))
                nc.vector.tensor_mul(D3[:], D3[:], D3[:])
                nc.vector.reduce_sum(D2[:],
                                     D3.rearrange("p (n c) -> p n c", c=3),
                                     axis=mybir.AxisListType.X)
                # MW = (d2 < 145) * 2^(j%8), one fused op
                wts = X2[:, 576:584].unsqueeze(1)
                wts = wts.broadcast_to([128, NP // 8, 8])
                nc.vector.scalar_tensor_tensor(
                    out=MW.rearrange("p (n b) -> p n b", b=8),
                    in0=D2.rearrange("p (n b) -> p n b", b=8),
                    scalar=LOOSE_CUT, in1=wts,
                    op0=mybir.AluOpType.is_lt,
                    op1=mybir.AluOpType.mult,
                )
                # byte n = sum_b bit_b * 2^b  (little-endian packbits;
                # sum <= 255 so u8 accumulation is exact)
                with nc.allow_low_precision(reason="u8 bitpack sum <= 255"):
                    pack = nc.vector.reduce_sum(
                        P3[:], MW.rearrange("p (n b) -> p n b", b=8),
                        axis=mybir.AxisListType.X)
                pack.then_inc(s_v, 1)
    return nc


def _get_program(repeat=1):
    key = ("nc", repeat)
    if key not in _CACHE:
        _CACHE[key] = _build_program(repeat)
    return _CACHE[key]


def _device_inputs(species, coordinates):
    """Per-core input maps: x2[p] = [full X of mol p//2 | its (p%2) i-half]."""
    nb = species > 0                                         # [512,128]
    Xm = np.ascontiguousarray(coordinates, dtype=np.float32).copy()
    aidx = np.arange(MOLSIZE, dtype=np.float32)
    blank = np.zeros((MOLSIZE, 3), np.float32)
    blank[:, 0] = 3.0e4 + 2.0e2 * aidx   # f16-safe, distinct per atom
    Xm[~nb] = np.broadcast_to(blank[None], Xm.shape)[~nb]

    H = MOLSIZE // 2
    wts = np.broadcast_to((2.0 ** np.arange(8, dtype=np.float32))[None],
                          (128, 8))
    in_maps = []
    for c in range(N_CORES):
        Xc = Xm[c * MPC:(c + 1) * MPC]                       # [64,128,3]
        full = np.repeat(Xc.reshape(MPC, MOLSIZE * 3), 2, axis=0)  # [128,384]
        halves = Xc.reshape(MPC, 2, H * 3).reshape(128, H * 3)     # [128,192]
        x2 = np.concatenate([full, halves, wts], axis=1)     # [128,584]
        in_maps.append({"x2": np.ascontiguousarray(x2, dtype=np.float16)})
    return in_maps


def _run_device(species, coordinates, trace=False, repeat=1):
    from concourse.bass_utils import run_bass_kernel_spmd

    nc = _get_program(repeat)
    in_maps = _device_inputs(species, coordinates)
    res = run_bass_kernel_spmd(nc, in_maps, list(range(N_CORES)), trace=trace)
    _CACHE["last_res"] = res
    H = MOLSIZE // 2
    parts = []
    for c in range(N_CORES):
        pk = res.results[c]["pk"]                            # [128, 1024]
        bits = np.unpackbits(pk.reshape(MPC, 2, H, MOLSIZE // 8),
                             axis=-1, bitorder="little")     # [64,2,64,128]
        parts.append(bits.reshape(MPC, MOLSIZE, MOLSIZE))    # [64,i,j]
    cand = np.concatenate(parts, axis=0)                     # [512,128,128]
    return cand, res


def kernel(species, coordinates, tot_charge, tore, _trace=False):
    species = np.asarray(species)
    coordinates = np.ascontiguousarray(np.asarray(coordinates,
                                                  dtype=np.float32))
    tot_charge = np.asarray(tot_charge)
    tore = np.asarray(tore, dtype=np.float32)

    nmol, molsize = species.shape
    sp32 = species.astype(np.int32)

    cand, _ = _run_device(sp32, coordinates, trace=_trace)

    # ---- host post-processing (identical to reference, validated bitwise) ----
    nonblank = sp32 > 0
    flat_nb = nonblank.ravel()
    real_atoms = np.nonzero(flat_nb)[0].astype(np.int32)
    n_real = len(real_atoms)
    inv_real_atoms = np.zeros(nmol * molsize, np.int32)
    inv_real_atoms[real_atoms] = np.arange(n_real, dtype=np.int32)
    Z = sp32.ravel()[real_atoms]

    nHeavy = (sp32 > 1).sum(1).astype(np.int32)
    sp = sp32
    nSuperHeavy = (((sp > 12) & (sp < 18)) | ((sp > 20) & (sp < 30))
                   | ((sp > 32) & (sp < 36)) | ((sp > 38) & (sp < 48))
                   | ((sp > 50) & (sp < 54)) | ((sp > 70) & (sp < 80))
                   | (sp == 57)).sum(1).astype(np.int32)
    nHydro = (sp == 1).sum(1).astype(np.int32)

    n_charge = tore[sp32].sum(axis=1, dtype=np.float32).astype(np.int32) \
        - tot_charge.astype(np.int32)
    nocc = n_charge // 2

    t1 = (np.arange(molsize) * (molsize + 1)).reshape(1, -1)
    t2 = (np.arange(nmol) * molsize ** 2).reshape(-1, 1)
    maskd = (t1 + t2).ravel()[real_atoms].astype(np.int32)
    atom_molid = np.repeat(np.arange(nmol, dtype=np.int32), molsize)[flat_nb]

    # exact refilter of device candidates (device grid includes i>=j)
    mi, ii, jj = np.nonzero(cand)
    tri = ii < jj
    mi = mi[tri]; ii = ii[tri]; jj = jj[tri]
    ci = coordinates[mi, ii]
    cj = coordinates[mi, jj]
    pc = cj - ci
    p = pc * pc
    d2 = (p[:, 0] + p[:, 1]) + p[:, 2]
    keep = (d2 < np.float32(OUTERCUTOFF2)) \
        & flat_nb[mi * molsize + ii] & flat_nb[mi * molsize + jj]
    mi = mi[keep]; ii = ii[keep]; jj = jj[keep]; pc = pc[keep]; p = p[keep]

    pairdist = np.sqrt((p[:, 0] + p[:, 1]) + p[:, 2]).astype(np.float32)
    rij = pairdist * np.float32(LCF)
    xij = pc / pairdist[:, None]

    pair_first_val = (mi * molsize + ii).astype(np.int32)
    pair_second_val = (mi * molsize + jj).astype(np.int32)
    idxi = inv_real_atoms[pair_first_val]
    idxj = inv_real_atoms[pair_second_val]
    ni = Z[idxi]
    nj = Z[idxj]
    mask_o = (real_atoms[idxi] * molsize
              + real_atoms[idxj] % molsize).astype(np.int32)
    mask_l = (real_atoms[idxj] * molsize
              + real_atoms[idxi] % molsize).astype(np.int32)
    pair_molid = atom_molid[idxi]

    return (nmol, molsize, nSuperHeavy, nHeavy, nHydro, nocc, Z, maskd,
            atom_molid, mask_o, mask_l, pair_molid, ni, nj, idxi, idxj,
            xij, rij)
